# revision 1
# baseline (speedup 1.0000x reference)
"""Bahdanau additive attention on 8 Trainium2 NeuronCores.

Math (per batch b):
    dec_f  = decoder_hidden @ W_h                     [H]
    enc_f  = encoder_outputs[b] @ W_s                 [S, H]
    energy = tanh(dec_f + enc_f) @ v + addmask        [S]
    attn   = softmax(energy)                          [S]
    context= attn @ encoder_outputs[b]                [2H]

Sharding: data-parallel over batch, 8 batches per core, weights replicated.

Device layout choice: everything runs in "transposed" space. The host
pre-transposes encoder_outputs to encT[b] = enc[b].T (shape [2H, S], bf16) so
the feature (contraction) dim of the big matmul lands on SBUF partitions.
Per batch: the main matmul computes enc_f.T tiles [k=128, s=512] with W_s
tiles stationary (each stationary tile serves 2 matmuls); tanh runs on the
scalar engine straight out of PSUM with dec_f as a per-partition bias;
energy = v.T @ hidden accumulates via M=1 matmuls into one PSUM bank (the 4
s-chunks packed on partitions 0/32/64/96); the 0/-1e10 source mask is added
with a K=1 matmul; softmax runs along the free dim (exp with fused
accumulate, cross-partition total via a K=97 matmul against a 0/1 selector,
1/sum re-broadcast with a K=1 matmul); attn is broadcast across partitions by
an SWDGE DMA re-reading the DRAM output row with a stride-0 leading dim;
context is a free-dim weighted reduction (DVE multiply + free-dim sum) over
the same encT tiles, so encoder_outputs is read from HBM exactly once.
Batch b's softmax/broadcast/context are emitted inside batch b+1's main loop
so the in-order PE queue never waits on the ACT/DVE chain.
"""

import numpy as np
import ml_dtypes

import concourse.bacc as bacc
import concourse.mybir as mybir
import concourse.tile as tile
from concourse.bass_utils import run_bass_kernel_spmd

# Problem shapes (hardcoded per contest rules).
B, S, H = 64, 2048, 1024
E = 2 * H            # encoder feature dim
NC = 8               # cores
BPC = B // NC        # batches per core
P = 128              # partitions
ET = E // P          # 16 e-tiles (contraction tiles of main matmul)
KT = H // P          # 8 k-tiles (hidden dim tiles)
SC = S // 512        # 4 s-chunks of 512
NEG_BIG = -1e10

F32 = mybir.dt.float32
BF16 = mybir.dt.bfloat16

_CACHE = {}


def _build():
    nc = bacc.Bacc("TRN2", target_bir_lowering=False, debug=False, num_devices=NC)

    encT_d = nc.dram_tensor("encT", [BPC, E, S], BF16, kind="ExternalInput")
    ws_d = nc.dram_tensor("wsT", [E, H], BF16, kind="ExternalInput")
    wh_d = nc.dram_tensor("whT", [H, H], BF16, kind="ExternalInput")
    dh_d = nc.dram_tensor("dhT", [H, BPC], BF16, kind="ExternalInput")
    v_d = nc.dram_tensor("vv", [P, KT], BF16, kind="ExternalInput")
    am_d = nc.dram_tensor("amask", [BPC, S], BF16, kind="ExternalInput")
    # natural-layout copy of the LAST local batch (for the tail fast path)
    encN_d = nc.dram_tensor("encN", [S, E], BF16, kind="ExternalInput")

    attn_d = nc.dram_tensor("attn", [BPC, S], F32, kind="ExternalOutput")
    ctx_d = nc.dram_tensor("ctxr", [P, BPC * ET], F32, kind="ExternalOutput")
    ctxl_d = nc.dram_tensor("ctxl", [1, E], F32, kind="ExternalOutput")

    with tile.TileContext(nc) as tc:
        with (
            tc.tile_pool(name="const", bufs=1) as cpool,
            tc.tile_pool(name="psum_mm", bufs=7, space="PSUM") as mmp,
            tc.tile_pool(name="psum_en", bufs=1, space="PSUM") as enp,
        ):
            # ---- persistent constants ----
            v_sb = cpool.tile([P, KT], BF16)
            ws_sb = cpool.tile([P, ET, H], BF16)
            decf_sb = cpool.tile([P, KT, BPC], F32)
            one_one = cpool.tile([1, 1], BF16)
            nc.vector.memset(one_one[:], 1.0)
            ones_row = cpool.tile([1, P], F32)
            nc.vector.memset(ones_row[:], 1.0)
            # energy lives packed on partitions {0,32,64,96} of ONE psum
            # bank; ones4 selects those rows in the cross-partition sum
            # matmul, ssum_t holds the per-chunk exp partial sums.
            ones4 = cpool.tile([97, 1], F32)
            nc.vector.memset(ones4[:], 0.0)
            ssum_t = cpool.tile([97, 1], F32)
            nc.vector.memset(ssum_t[:], 0.0)
            for c in range(SC):
                nc.vector.memset(ones4[32 * c:32 * c + 1, :], 1.0)
            ctx_acc = cpool.tile([P, BPC * ET], F32)

            # ---- batch pipeline ----
            with (
                tc.tile_pool(name="encp", bufs=8) as encp,
                tc.tile_pool(name="work", bufs=2) as wkp,
                tc.tile_pool(name="dscr", bufs=1, space="DRAM") as dscr,
            ):
                # startup: dec_f = (decoder_hidden @ W_h).T -> [k, b].
                # W_h borrows one encq slot (same 16KB/partition footprint).
                # Small DMAs first so the PE's dec_f matmuls start early,
                # W_s in two k-halves so batch 0's first k-tiles aren't
                # gated on the full weight load.
                nc.sync.dma_start(v_sb[:], v_d.ap())
                wh_sb = encp.tile([P, KT, H], BF16, tag="encq")
                nc.sync.dma_start(wh_sb[:], wh_d.ap().rearrange("(t p) k -> p t k", p=P))
                dh_sb = wkp.tile([P, KT, BPC], BF16, tag="dh", bufs=1)
                nc.sync.dma_start(dh_sb[:], dh_d.ap().rearrange("(t p) b -> p t b", p=P))
                ws_ap = ws_d.ap().rearrange("(t p) k -> p t k", p=P)
                nc.sync.dma_start(ws_sb[:, :, 0:H // 2], ws_ap[:, :, 0:H // 2])

                def emit_decf(ks):
                    # dec_f matmuls. k=0 is emitted before the batch loop (its
                    # tanh consumers in batch 0 need a recorded writer);
                    # k=1..7 are injected after batch 0's first k-tile so the
                    # PE starts on the big matmul as soon as data lands.
                    # PSUM->SBUF copies go on DVE: ACT's in-order queue
                    # already holds batch-0 tanhs that consume decf_sb.
                    for k in ks:
                        dps = mmp.tile([P, BPC], F32, tag="mm", name=f"dps_{k}")
                        for h in range(KT):
                            nc.tensor.matmul(
                                dps[:],
                                wh_sb[:, h, k * P:(k + 1) * P],
                                dh_sb[:, h, :],
                                start=(h == 0),
                                stop=(h == KT - 1),
                            )
                        nc.vector.tensor_copy(decf_sb[:, k, :], dps[:])

                emit_decf(range(KT))

                state = {}

                def emit_load(b):
                    quarters = []
                    for q in range(4):
                        encq = encp.tile([P, 4, S], BF16, tag="encq", name=f"encq_{b}_{q}")
                        nc.sync.dma_start(
                            encq[:],
                            encT_d.ap()[b, q * 512:(q + 1) * 512, :].rearrange(
                                "(t p) s -> p t s", p=P
                            ),
                        )
                        quarters.append(encq)
                    mask_st = wkp.tile([1, S], BF16, tag="mask", bufs=1, name=f"mask_{b}")
                    nc.sync.dma_start(mask_st[:], am_d.ap()[b:b + 1, :])
                    state[b] = (quarters, mask_st)

                def emit_exp(b):
                    """Exp over batch b's energy PSUM (4 packed chunk rows).
                    Emitted at the start of batch b+1 so the single
                    energy-PSUM buffer frees before b+1 needs it."""
                    energy_ps = state[b, "energy"]
                    exps = wkp.tile([97, 512], F32, tag="exps", bufs=1,
                                    name=f"exps_{b}")
                    for c in range(SC):
                        nc.scalar.activation(
                            exps[32 * c:32 * c + 1, :],
                            energy_ps[32 * c:32 * c + 1, :],
                            mybir.ActivationFunctionType.Exp,
                            accum_out=ssum_t[32 * c:32 * c + 1, :],
                        )
                    state[b, "exp"] = exps

                def emit_sum(b):
                    """Cross-partition exp total + reciprocal for batch b.
                    Emitted one k-tile before emit_post so the PE never waits
                    on the DVE reciprocal between the two tiny matmuls."""
                    sum_ps = mmp.tile([1, 1], F32, tag="mm", name=f"sum_ps_{b}")
                    nc.tensor.matmul(sum_ps[:], ssum_t[:], ones4[:],
                                     start=True, stop=True)
                    sinv = wkp.tile([1, 1], F32, tag="sinv", name=f"sinv_{b}")
                    nc.vector.reciprocal(sinv[:], sum_ps[:])
                    state[b, "sinv"] = sinv

                def emit_post(b, last=False):
                    """Softmax tail + attn broadcast + context for batch b.

                    Emitted in the middle of batch b+1's main loop so the PE
                    never stalls waiting on the ACT/DVE softmax chain."""
                    quarters, _ = state[b]
                    exps = state[b, "exp"]
                    sinv = state[b, "sinv"]
                    attn_bc = wkp.tile([P, S], BF16, tag="attn_bc", name=f"attn_bc_{b}")
                    if last:
                        # tail fast path: broadcast UNNORMALIZED bf16 exps via
                        # a DRAM scratch row with HWDGE (no SWDGE drain, no
                        # wait on the normalization chain); 1/sum is folded
                        # into the final context columns instead.
                        exps_bf = wkp.tile([97, 512], BF16, tag="exps_bf", bufs=1,
                                           name=f"exps_bf_{b}")
                        nc.vector.tensor_copy(exps_bf[:], exps[:])
                        bsc = dscr.tile([1, S], BF16, name=f"bsc_{b}")
                        for c in range(SC):
                            nc.sync.dma_start(
                                bsc[0:1, c * 512:(c + 1) * 512],
                                exps_bf[32 * c:32 * c + 1, :],
                            )
                        nc.sync.dma_start(
                            attn_bc[:], bsc[0:1, :].broadcast_to((P, S))
                        )
                    # broadcast 1/sum to all partitions with a K=1 matmul
                    sb_ps = mmp.tile([P, 1], F32, tag="mm", name=f"sb_ps_{b}")
                    nc.tensor.matmul(sb_ps[:], ones_row[:], sinv[:],
                                     start=True, stop=True)
                    sinv_all = wkp.tile([P, 1], F32, tag="sinv_all",
                                        name=f"sinv_all_{b}")
                    nc.vector.tensor_copy(sinv_all[:], sb_ps[:])
                    attn_row = wkp.tile([97, 512], F32, tag="attn_row", bufs=1,
                                        name=f"attn_row_{b}")
                    nc.vector.tensor_scalar_mul(attn_row[:], exps[:],
                                                sinv_all[0:97, :])
                    for c in range(SC):
                        nc.sync.dma_start(
                            attn_d.ap()[b:b + 1, c * 512:(c + 1) * 512],
                            attn_row[32 * c:32 * c + 1, :],
                        )
                    if not last:
                        # pipelined path: broadcast the normalized attn row
                        # from its DRAM output slot (SWDGE: cast in flight)
                        nc.gpsimd.dma_start(
                            attn_bc[:], attn_d.ap()[b:b + 1, :].broadcast_to((P, S))
                        )

                    if last:
                        # tail fast path: context on the (now idle) PE.
                        # attn columns come from the bf16 scratch row via a
                        # DMA xbar transpose [16,128] -> [128,16]; the 64
                        # matmuls accumulate unnormalized context into the
                        # packed one-bank PSUM layout; 1/sum is applied by
                        # the scalar engine during the PSUM->SBUF copy.
                        attnT = wkp.tile([P, ET], BF16, tag="attnT", bufs=1,
                                         name=f"attnT_{b}")
                        nc.sync.dma_start_transpose(
                            attnT[:],
                            bsc[0:1, :].rearrange("o (t p) -> (o t) p", p=P),
                        )
                        natq = state["natq"]
                        ctxps = enp.tile([97, 512], F32, tag="energy",
                                         name="ctxps_last")
                        for t in range(ET):
                            for c in range(SC):
                                nc.tensor.matmul(
                                    ctxps[32 * c:32 * c + 1, :],
                                    attnT[:, t:t + 1],
                                    natq[t // 4][:, t % 4, c * 512:(c + 1) * 512],
                                    start=(t == 0),
                                    stop=(t == ET - 1),
                                    skip_group_check=True,
                                    tile_position=(0, 32 * c),
                                )
                        ctx_row = wkp.tile([97, 512], F32, tag="ctx_row", bufs=1,
                                           name="ctx_row_last")
                        for c in range(SC):
                            nc.scalar.activation(
                                ctx_row[32 * c:32 * c + 1, :],
                                ctxps[32 * c:32 * c + 1, :],
                                mybir.ActivationFunctionType.Copy,
                                scale=sinv_all[32 * c:32 * c + 1, :],
                            )
                            nc.sync.dma_start(
                                ctxl_d.ap()[0:1, c * 512:(c + 1) * 512],
                                ctx_row[32 * c:32 * c + 1, :],
                            )
                        return
                    # context: DVE multiply + free-dim reduce over encT tiles
                    for e in range(ET):
                        q, qt = divmod(e, 4)
                        scr = wkp.tile([P, S], BF16, tag="scr", bufs=4, name=f"scr_{b}_{e}")
                        nc.vector.tensor_mul(scr[:], quarters[q][:, qt, :], attn_bc[:])
                        acc = ctx_acc[:, b * ET + e:b * ET + e + 1]
                        nc.vector.reduce_sum(acc, scr[:], axis=mybir.AxisListType.X)

                # batch 0's data next on the DMA queue, then the rest of W_s
                emit_load(0)
                nc.sync.dma_start(ws_sb[:, :, H // 2:H], ws_ap[:, :, H // 2:H])
                emit_load(1)
                for b in range(BPC):
                    quarters, mask_st = state[b]
                    if b > 0:
                        emit_exp(b - 1)
                    energy_ps = enp.tile([97, 512], F32, tag="energy", name=f"energy_{b}")
                    state[b, "energy"] = energy_ps

                    # main matmul: enc_f.T tiles + tanh + energy accumulation.
                    # s-chunks paired inside the e-loop so each stationary
                    # W_s tile serves 2 matmuls. Energy matmuls for each half
                    # are deferred one half-iteration so the PE never waits
                    # on the tanh that produces their hidden input.
                    pending = []

                    def flush_pending(keep=0):
                        while len(pending) > keep:
                            k_, sc_, hid_ = pending.pop(0)
                            nc.tensor.matmul(
                                energy_ps[32 * sc_:32 * sc_ + 1, :],
                                v_sb[:, k_:k_ + 1],
                                hid_[:],
                                start=(k_ == 0),
                                stop=False,
                                skip_group_check=True,
                                tile_position=(0, 32 * sc_),
                            )

                    for k in range(KT):
                        if k == 1 and b > 0:
                            emit_sum(b - 1)
                        if k == 2 and b > 0:
                            emit_post(b - 1)
                        if k == 4 and b + 2 < BPC:
                            emit_load(b + 2)
                        if k == 5 and b == BPC - 1:
                            natq = []
                            for q in range(4):
                                nq = encp.tile([P, 4, E], BF16, tag="encq",
                                               name=f"natq_{q}")
                                nc.sync.dma_start(
                                    nq[:],
                                    encN_d.ap()[q * 512:(q + 1) * 512, :].rearrange(
                                        "(t p) e -> p t e", p=P
                                    ),
                                )
                                natq.append(nq)
                            state["natq"] = natq
                        for half in range(SC // 2):
                            pss = []
                            for j in range(2):
                                sc = half * 2 + j
                                ps = mmp.tile([P, 512], F32, tag="mm",
                                              name=f"ps_{b}_{k}_{sc}")
                                pss.append(ps)
                            for e in range(ET):
                                q, qt = divmod(e, 4)
                                for j in range(2):
                                    sc = half * 2 + j
                                    nc.tensor.matmul(
                                        pss[j][:],
                                        ws_sb[:, e, k * P:(k + 1) * P],
                                        quarters[q][:, qt, sc * 512:(sc + 1) * 512],
                                        start=(e == 0),
                                        stop=(e == ET - 1),
                                    )
                            flush_pending(keep=0)
                            for j in range(2):
                                sc = half * 2 + j
                                hid = wkp.tile([P, 512], BF16, tag="hid", bufs=8,
                                               name=f"hid_{b}_{k}_{sc}")
                                nc.scalar.activation(
                                    hid[:],
                                    pss[j][:],
                                    mybir.ActivationFunctionType.Tanh,
                                    bias=decf_sb[:, k, b:b + 1],
                                )
                                pending.append((k, sc, hid))
                    flush_pending()
                    # add the (0 / -1e10) mask via a K=1 matmul
                    for sc in range(SC):
                        nc.tensor.matmul(
                            energy_ps[32 * sc:32 * sc + 1, :],
                            one_one[:],
                            mask_st[0:1, sc * 512:(sc + 1) * 512],
                            start=False,
                            stop=True,
                            skip_group_check=True,
                            tile_position=(0, 32 * sc),
                        )
                emit_exp(BPC - 1)
                emit_sum(BPC - 1)
                emit_post(BPC - 1, last=True)

            nc.sync.dma_start(ctx_d.ap()[:], ctx_acc[:])

    nc.compile()
    return nc


def _prep_inputs(decoder_hidden, encoder_outputs, src_mask, W_h, W_s, v):
    bf = ml_dtypes.bfloat16
    encoder_outputs = np.asarray(encoder_outputs)
    encT = np.ascontiguousarray(encoder_outputs.transpose(0, 2, 1)).astype(bf)
    wsT = W_s.astype(bf)
    whT = W_h.astype(bf)
    dhT = np.ascontiguousarray(decoder_hidden.T).astype(bf)
    vv = np.ascontiguousarray(v.reshape(KT, P).T).astype(bf)
    amask = np.where(src_mask == 0, np.float32(NEG_BIG), np.float32(0.0)).astype(bf)

    in_maps = []
    for c in range(NC):
        lo, hi = c * BPC, (c + 1) * BPC
        in_maps.append({
            "encT": np.ascontiguousarray(encT[lo:hi]),
            "wsT": wsT,
            "whT": whT,
            "dhT": np.ascontiguousarray(dhT[:, lo:hi]),
            "vv": vv,
            "amask": np.ascontiguousarray(amask[lo:hi]),
            "encN": np.ascontiguousarray(encoder_outputs[hi - 1].astype(bf)),
        })
    return in_maps


def kernel(decoder_hidden, encoder_outputs, src_mask, W_h, W_s, v, _trace=False):
    if "nc" not in _CACHE:
        _CACHE["nc"] = _build()
    nc = _CACHE["nc"]

    in_maps = _prep_inputs(
        np.asarray(decoder_hidden, dtype=np.float32),
        np.asarray(encoder_outputs, dtype=np.float32),
        np.asarray(src_mask),
        np.asarray(W_h, dtype=np.float32),
        np.asarray(W_s, dtype=np.float32),
        np.asarray(v, dtype=np.float32),
    )

    res = run_bass_kernel_spmd(nc, in_maps, core_ids=list(range(NC)), trace=_trace)
    _CACHE["last_result"] = res

    context = np.empty((B, E), dtype=np.float32)
    attn = np.empty((B, S), dtype=np.float32)
    for c in range(NC):
        lo, hi = c * BPC, (c + 1) * BPC
        attn[lo:hi] = res.results[c]["attn"]
        raw = res.results[c]["ctxr"]  # [P, BPC*ET]
        context[lo:hi] = raw.reshape(P, BPC, ET).transpose(1, 2, 0).reshape(BPC, E)
        # last local batch's context comes from the PE tail fast path
        context[hi - 1] = res.results[c]["ctxl"][0]
    return context, attn



# revision 7
# speedup vs baseline: 1.7923x; 1.7923x over previous
"""Bahdanau additive attention on 8 Trainium2 NeuronCores.

Math (per batch b):
    dec_f  = decoder_hidden @ W_h                     [H]
    enc_f  = encoder_outputs[b] @ W_s                 [S, H]
    energy = tanh(dec_f + enc_f) @ v + addmask        [S]
    attn   = softmax(energy)                          [S]
    context= attn @ encoder_outputs[b]                [2H]

Sharding: data-parallel over batch, 8 batches per core, weights replicated.

Device layout choice: everything runs in "transposed" space — the feature
(contraction) dim of the big matmul lands on SBUF partitions. All DRAM
inputs are pre-swizzled partition-major on the host so every SBUF load is
one contiguous chunk per partition (1 DMA descriptor per partition).

The big matmul runs in fp8 (e4m3) with perf_mode=DoubleRow: the host ships
encT in fp8 and W_s pre-scaled by 256 in fp8 (so its +-1/32 entries use the
normal fp8 range); pairs of contraction tiles fuse into one 256-deep matmul,
halving PE time vs bf16 (measured 216 ns per [256x128]x[256x512] matmul,
the N/2.4GHz streaming floor). The 1/256 descale folds into the tanh's
scale operand. The context weighted-sum needs ~bf16-accurate encoder
values, so a second bf16 copy of encT streams in for the DVE: per batch,
context is a fused multiply+reduce (scalar_tensor_tensor with accum_out;
tensor_tensor_reduce wedges the device) against the SWDGE-broadcast attn
row. enc_f tiles [k=128, s=512] accumulate with W_s pair-tiles stationary
(each stationary serves 2 matmuls); tanh runs on the scalar engine straight
out of PSUM with dec_f as a per-partition bias; energy = v.T @ hidden
accumulates via M=1 matmuls into one of TWO alternating PSUM banks (the 4
s-chunks packed on partitions 0/32/64/96) so a batch never waits on the
previous batch's softmax reads; the 0/-1e10 source mask is added with a
K=1 matmul; softmax runs along the free dim. The previous batch's exp
chunks are emitted one-per-half during the first two k-tiles so they
interleave with tanhs in the scalar engine's strict FIFO instead of
stalling the v-dot matmuls at the batch boundary. Batch b's softmax/
broadcast/context are emitted inside batch b+1's main loop so the in-order
PE queue never waits on the ACT/DVE chain. The last local batch's context
runs on the (by then idle) PE against a natural-layout bf16 copy, with the
attn column vector built by 16 K=1 outer-product matmuls from SBUF (no
DRAM bounce, no transpose DMA).
"""

import numpy as np
import ml_dtypes

import concourse.bacc as bacc
import concourse.mybir as mybir
import concourse.tile as tile
from concourse.bass_utils import run_bass_kernel_spmd

# Problem shapes (hardcoded per contest rules).
B, S, H = 64, 2048, 1024
E = 2 * H            # encoder feature dim
NC = 8               # cores
BPC = B // NC        # batches per core
P = 128              # partitions
ET = E // P          # 16 e-tiles (contraction tiles of main matmul)
PT = ET // 2         # 8 DoubleRow pair-tiles
KT = H // P          # 8 k-tiles (hidden dim tiles)
SC = S // 512        # 4 s-chunks of 512
NEG_BIG = -1e10
WS_SCALE = 256.0     # fp8 pre-scale on W_s (power of 2; descaled in tanh)

F32 = mybir.dt.float32
BF16 = mybir.dt.bfloat16
F8 = mybir.dt.float8e4

_CACHE = {}


def _build():
    nc = bacc.Bacc("TRN2", target_bir_lowering=False, debug=False, num_devices=NC)

    # all layouts partition-major: one contiguous chunk per partition per load
    encT_d = nc.dram_tensor("encT", [BPC, 4, P, 4, S], F8, kind="ExternalInput")
    encB_d = nc.dram_tensor("encB", [BPC - 1, 4, P, 4, S], BF16, kind="ExternalInput")
    ws_d = nc.dram_tensor("wsT", [P, ET, H], F8, kind="ExternalInput")
    wh_d = nc.dram_tensor("whT", [P, KT, H], BF16, kind="ExternalInput")
    dh_d = nc.dram_tensor("dhT", [P, KT, BPC], BF16, kind="ExternalInput")
    v_d = nc.dram_tensor("vv", [P, KT], BF16, kind="ExternalInput")
    am_d = nc.dram_tensor("amask", [BPC, S], BF16, kind="ExternalInput")
    # natural-layout copy of the LAST local batch (for the tail fast path)
    encN_d = nc.dram_tensor("encN", [4, P, 4, E], BF16, kind="ExternalInput")

    attn_d = nc.dram_tensor("attn", [BPC, S], F32, kind="ExternalOutput")
    ctx_d = nc.dram_tensor("ctxr", [P, BPC * ET], F32, kind="ExternalOutput")
    ctxl_d = nc.dram_tensor("ctxl", [1, E], F32, kind="ExternalOutput")

    with tile.TileContext(nc) as tc:
        with (
            tc.tile_pool(name="const", bufs=1) as cpool,
            tc.tile_pool(name="psum_mm", bufs=6, space="PSUM") as mmp,
            tc.tile_pool(name="psum_en", bufs=2, space="PSUM") as enp,
        ):
            # ---- persistent constants ----
            v_sb = cpool.tile([P, KT], BF16)
            ws_sb = cpool.tile([P, ET, H], F8)
            decf_sb = cpool.tile([P, KT, BPC], F32)
            one_one = cpool.tile([1, 1], BF16)
            nc.vector.memset(one_one[:], 1.0)
            ones_row = cpool.tile([1, P], F32)
            nc.vector.memset(ones_row[:], 1.0)
            # energy lives packed on partitions {0,32,64,96} of one psum
            # bank; ones4 selects those rows in the cross-partition sum
            # matmul, ssum_t holds the per-chunk exp partial sums, and
            # ones4_bf streams the K=1 outer products of the tail's attnT.
            ones4 = cpool.tile([97, 1], F32)
            nc.vector.memset(ones4[:], 0.0)
            ones4_bf = cpool.tile([97, 1], BF16)
            nc.vector.memset(ones4_bf[:], 1.0)
            ssum_t = cpool.tile([97, 1], F32)
            nc.vector.memset(ssum_t[:], 0.0)
            for c in range(SC):
                nc.vector.memset(ones4[32 * c:32 * c + 1, :], 1.0)
            ctx_acc = cpool.tile([P, BPC * ET], F32)
            # last batch's columns are produced by the PE tail path instead
            nc.vector.memset(ctx_acc[:], 0.0)

            # ---- batch pipeline ----
            with (
                tc.tile_pool(name="encp", bufs=8) as encp,
                tc.tile_pool(name="encbp", bufs=5) as encbp,
                tc.tile_pool(name="work", bufs=2) as wkp,
            ):
                # startup: dec_f = (decoder_hidden @ W_h).T -> [k, b].
                # W_h borrows one bf16-stream slot (same 16KB/partition
                # footprint). Small DMAs first so the PE's dec_f matmuls
                # start early.
                nc.sync.dma_start(v_sb[:], v_d.ap())
                wh_sb = encbp.tile([P, KT, H], BF16, tag="encb")
                nc.sync.dma_start(wh_sb[:], wh_d.ap())
                dh_sb = wkp.tile([P, KT, BPC], BF16, tag="dh", bufs=1)
                nc.sync.dma_start(dh_sb[:], dh_d.ap())
                nc.sync.dma_start(ws_sb[:], ws_d.ap())

                def emit_decf(ks):
                    # dec_f matmuls. PSUM->SBUF copies go on DVE: ACT's
                    # in-order queue already holds batch-0 tanhs that
                    # consume decf_sb.
                    for k in ks:
                        dps = mmp.tile([P, BPC], F32, tag="mm", name=f"dps_{k}")
                        for h in range(KT):
                            nc.tensor.matmul(
                                dps[:],
                                wh_sb[:, h, k * P:(k + 1) * P],
                                dh_sb[:, h, :],
                                start=(h == 0),
                                stop=(h == KT - 1),
                            )
                        nc.vector.tensor_copy(decf_sb[:, k, :], dps[:])

                emit_decf(range(KT))

                state = {}

                def emit_load(b):
                    quarters = []
                    for q in range(4):
                        encq = encp.tile([P, 4, S], F8, tag="encq", name=f"encq_{b}_{q}")
                        nc.sync.dma_start(encq[:], encT_d.ap()[b, q])
                        quarters.append(encq)
                    mask_st = wkp.tile([1, S], BF16, tag="mask", bufs=1, name=f"mask_{b}")
                    nc.sync.dma_start(mask_st[:], am_d.ap()[b:b + 1, :])
                    state[b] = (quarters, mask_st)

                def emit_load_bf(b):
                    # bf16 copy of batch b's encT for the context reduction
                    # (consumed during batch b+1). The last batch skips it:
                    # its context runs on the PE from the natural copy.
                    bfq = []
                    for q in range(4):
                        t = encbp.tile([P, 4, S], BF16, tag="encb", name=f"encb_{b}_{q}")
                        nc.sync.dma_start(t[:], encB_d.ap()[b, q])
                        bfq.append(t)
                    state[b, "bf"] = bfq

                def emit_exp_chunk(b, c):
                    """Exp over one of batch b's energy chunk rows. Chunks
                    are emitted one per half-iteration of batch b+1 so they
                    interleave with tanhs in ACT's strict FIFO instead of
                    blocking the v-dot matmuls at the batch boundary."""
                    energy_ps = state[b, "energy"]
                    if c == 0:
                        exps = wkp.tile([97, 512], F32, tag="exps", bufs=1,
                                        name=f"exps_{b}")
                        # rows between the 4 chunk strips are never written;
                        # zero them so downstream full-height reads are defined
                        nc.vector.memset(exps[:], 0.0)
                        state[b, "exp"] = exps
                    exps = state[b, "exp"]
                    nc.scalar.activation(
                        exps[32 * c:32 * c + 1, :],
                        energy_ps[32 * c:32 * c + 1, :],
                        mybir.ActivationFunctionType.Exp,
                        accum_out=ssum_t[32 * c:32 * c + 1, :],
                    )

                def emit_sum(b):
                    """Cross-partition exp total + reciprocal for batch b.
                    Emitted one k-tile before emit_post so the PE never waits
                    on the DVE reciprocal between the two tiny matmuls."""
                    sum_ps = mmp.tile([1, 1], F32, tag="mm", name=f"sum_ps_{b}")
                    nc.tensor.matmul(sum_ps[:], ssum_t[:], ones4[:],
                                     start=True, stop=True)
                    sinv = wkp.tile([1, 1], F32, tag="sinv", name=f"sinv_{b}")
                    nc.vector.reciprocal(sinv[:], sum_ps[:])
                    state[b, "sinv"] = sinv

                def emit_post(b, last=False):
                    """Softmax tail + attn broadcast + context for batch b.

                    Emitted in the middle of batch b+1's main loop so the PE
                    never stalls waiting on the ACT/DVE softmax chain."""
                    exps = state[b, "exp"]
                    sinv = state[b, "sinv"]
                    # broadcast 1/sum to all partitions with a K=1 matmul
                    sb_ps = mmp.tile([P, 1], F32, tag="mm", name=f"sb_ps_{b}")
                    nc.tensor.matmul(sb_ps[:], ones_row[:], sinv[:],
                                     start=True, stop=True)
                    sinv_all = wkp.tile([P, 1], F32, tag="sinv_all",
                                        name=f"sinv_all_{b}")
                    nc.vector.tensor_copy(sinv_all[:], sb_ps[:])
                    attn_row = wkp.tile([97, 512], F32, tag="attn_row", bufs=1,
                                        name=f"attn_row_{b}")
                    nc.vector.tensor_scalar_mul(attn_row[:], exps[:],
                                                sinv_all[0:97, :])
                    # one DMA for the whole attn row (partition-strided src)
                    nc.sync.dma_start(
                        attn_d.ap()[b:b + 1, :].rearrange("o (c x) -> (o c) x", c=SC),
                        attn_row[0:97:32, :],
                    )

                    if last:
                        # tail fast path: context on the (now idle) PE.
                        # attnT columns are built by 16 K=1 outer-product
                        # matmuls straight from SBUF (exps_bf row segment x
                        # scalar 1) into one PSUM bank -- no DRAM bounce.
                        # The 64 context matmuls then accumulate the
                        # unnormalized context into the packed one-bank
                        # PSUM layout; 1/sum is applied by the scalar
                        # engine during the PSUM->SBUF copy.
                        exps_bf = wkp.tile([97, 512], BF16, tag="exps_bf", bufs=1,
                                           name=f"exps_bf_{b}")
                        nc.vector.tensor_copy(exps_bf[:], exps[:])
                        atps = mmp.tile([P, ET], F32, tag="mm", name="atps")
                        for t in range(ET):
                            c, seg = divmod(t, 4)
                            nc.tensor.matmul(
                                atps[:, t:t + 1],
                                exps_bf[32 * c:32 * c + 1, seg * P:(seg + 1) * P],
                                ones4_bf[32 * c:32 * c + 1, :],
                                start=(t == 0),
                                stop=(t == ET - 1),
                                skip_group_check=True,
                                tile_position=(32 * c, 0),
                            )
                        attnT = wkp.tile([P, ET], BF16, tag="attnT", bufs=1,
                                         name=f"attnT_{b}")
                        nc.vector.tensor_copy(attnT[:], atps[:])
                        natq = state["natq"]
                        ctxps = enp.tile([97, 512], F32, tag="energy",
                                         name="ctxps_last")
                        for t in range(ET):
                            for c in range(SC):
                                nc.tensor.matmul(
                                    ctxps[32 * c:32 * c + 1, :],
                                    attnT[:, t:t + 1],
                                    natq[t // 4][:, t % 4, c * 512:(c + 1) * 512],
                                    start=(t == 0),
                                    stop=(t == ET - 1),
                                    skip_group_check=True,
                                    tile_position=(0, 32 * c),
                                )
                        ctx_row = wkp.tile([97, 512], F32, tag="ctx_row", bufs=1,
                                           name="ctx_row_last")
                        for c in range(SC):
                            nc.scalar.activation(
                                ctx_row[32 * c:32 * c + 1, :],
                                ctxps[32 * c:32 * c + 1, :],
                                mybir.ActivationFunctionType.Copy,
                                scale=sinv_all[32 * c:32 * c + 1, :],
                            )
                        nc.sync.dma_start(
                            ctxl_d.ap()[0:1, :].rearrange("o (c x) -> (o c) x", c=SC),
                            ctx_row[0:97:32, :],
                        )
                        return
                    # pipelined path: broadcast the normalized attn row
                    # from its DRAM output slot (SWDGE: cast in flight)
                    attn_bc = wkp.tile([P, S], BF16, tag="attn_bc", name=f"attn_bc_{b}")
                    nc.gpsimd.dma_start(
                        attn_bc[:], attn_d.ap()[b:b + 1, :].broadcast_to((P, S))
                    )
                    # context: fused multiply+reduce over the bf16 encT tiles
                    bfq = state[b, "bf"]
                    for e in range(ET):
                        q, qt = divmod(e, 4)
                        scr = wkp.tile([P, S], BF16, tag="scr", bufs=2, name=f"scr_{b}_{e}")
                        acc = ctx_acc[:, b * ET + e:b * ET + e + 1]
                        nc.vector.scalar_tensor_tensor(
                            out=scr[:],
                            in0=bfq[q][:, qt, :],
                            scalar=1.0,
                            in1=attn_bc[:],
                            op0=mybir.AluOpType.mult,
                            op1=mybir.AluOpType.mult,
                            accum_out=acc,
                        )

                # batch 0's data next on the DMA queue
                emit_load(0)
                emit_load(1)
                for b in range(BPC):
                    quarters, mask_st = state[b]
                    energy_ps = enp.tile([97, 512], F32, tag="energy", name=f"energy_{b}")
                    state[b, "energy"] = energy_ps

                    # main matmul: enc_f.T tiles + tanh + energy accumulation.
                    # fp8 DoubleRow fuses e-tile pairs into 256-deep matmuls;
                    # s-chunks paired inside the pair-loop so each stationary
                    # W_s pair-tile serves 2 matmuls. Energy matmuls for each
                    # half are deferred one half-iteration so the PE never
                    # waits on the tanh that produces their hidden input.
                    pending = []

                    def flush_pending(keep=0):
                        while len(pending) > keep:
                            k_, sc_, hid_ = pending.pop(0)
                            nc.tensor.matmul(
                                energy_ps[32 * sc_:32 * sc_ + 1, :],
                                v_sb[:, k_:k_ + 1],
                                hid_[:],
                                start=(k_ == 0),
                                stop=False,
                                skip_group_check=True,
                                tile_position=(0, 32 * sc_),
                            )

                    for k in range(KT):
                        if k == 1 and b < BPC - 1:
                            emit_load_bf(b)
                        if k == 2 and b > 0:
                            emit_sum(b - 1)
                        if k == 3 and b > 0:
                            emit_post(b - 1)
                        if k == 4 and b + 2 < BPC:
                            emit_load(b + 2)
                        if k == 3 and b == BPC - 1:
                            natq = []
                            for q in range(4):
                                nq = encbp.tile([P, 4, E], BF16, tag="encb",
                                                name=f"natq_{q}")
                                nc.sync.dma_start(nq[:], encN_d.ap()[q])
                                natq.append(nq)
                            state["natq"] = natq
                        if k == 4 and b == BPC - 1:
                            # the DVE context accumulator is complete once
                            # batch BPC-2's reduction (emitted at k==3) runs
                            nc.sync.dma_start(ctx_d.ap()[:], ctx_acc[:])
                        for half in range(SC // 2):
                            pss = []
                            for j in range(2):
                                sc = half * 2 + j
                                ps = mmp.tile([P, 512], F32, tag="mm",
                                              name=f"ps_{b}_{k}_{sc}")
                                pss.append(ps)
                            for t in range(PT):
                                q, qt = divmod(2 * t, 4)
                                for j in range(2):
                                    sc = half * 2 + j
                                    nc.tensor.matmul(
                                        pss[j][:],
                                        ws_sb[:, 2 * t:2 * t + 2, k * P:(k + 1) * P],
                                        quarters[q][:, qt:qt + 2,
                                                    sc * 512:(sc + 1) * 512],
                                        start=(t == 0),
                                        stop=(t == PT - 1),
                                        perf_mode=mybir.MatmulPerfMode.DoubleRow,
                                    )
                            flush_pending(keep=0)
                            for j in range(2):
                                sc = half * 2 + j
                                hid = wkp.tile([P, 512], BF16, tag="hid", bufs=8,
                                               name=f"hid_{b}_{k}_{sc}")
                                nc.scalar.activation(
                                    hid[:],
                                    pss[j][:],
                                    mybir.ActivationFunctionType.Tanh,
                                    bias=decf_sb[:, k, b:b + 1],
                                    scale=1.0 / WS_SCALE,
                                )
                                pending.append((k, sc, hid))
                            if b > 0 and k < 2:
                                emit_exp_chunk(b - 1, k * 2 + half)
                    flush_pending()
                    # add the (0 / -1e10) mask via a K=1 matmul
                    for sc in range(SC):
                        nc.tensor.matmul(
                            energy_ps[32 * sc:32 * sc + 1, :],
                            one_one[:],
                            mask_st[0:1, sc * 512:(sc + 1) * 512],
                            start=False,
                            stop=True,
                            skip_group_check=True,
                            tile_position=(0, 32 * sc),
                        )
                for c in range(SC):
                    emit_exp_chunk(BPC - 1, c)
                emit_sum(BPC - 1)
                emit_post(BPC - 1, last=True)

    nc.compile()
    return nc


def _prep_inputs(decoder_hidden, encoder_outputs, src_mask, W_h, W_s, v):
    bf = ml_dtypes.bfloat16
    f8 = ml_dtypes.float8_e4m3
    encoder_outputs = np.asarray(encoder_outputs)
    # [B, E, S] -> partition-major [B, 4(q), 128(p), 4(et), S]
    encT = np.ascontiguousarray(encoder_outputs.transpose(0, 2, 1))
    encT = encT.reshape(B, 4, 4, P, S).transpose(0, 1, 3, 2, 4)
    encT8 = np.ascontiguousarray(encT.astype(f8))
    encTb = np.ascontiguousarray(encT.astype(bf))
    # W_s [E, H] -> [128(p), 16(et), H], pre-scaled for fp8
    wsT = np.ascontiguousarray(
        (W_s * np.float32(WS_SCALE)).reshape(ET, P, H).transpose(1, 0, 2).astype(f8))
    # W_h [H, H] -> [128(p), 8(t), H]
    whT = np.ascontiguousarray(W_h.reshape(KT, P, H).transpose(1, 0, 2).astype(bf))
    # decoder_hidden.T [H, B] -> per-core [128(p), 8(t), BPC] below
    dhT = np.ascontiguousarray(decoder_hidden.T).astype(bf)
    vv = np.ascontiguousarray(v.reshape(KT, P).T).astype(bf)
    amask = np.where(src_mask == 0, np.float32(NEG_BIG), np.float32(0.0)).astype(bf)

    in_maps = []
    for c in range(NC):
        lo, hi = c * BPC, (c + 1) * BPC
        dhc = dhT[:, lo:hi].reshape(KT, P, BPC).transpose(1, 0, 2)
        encN = encoder_outputs[hi - 1].astype(bf)  # [S, E]
        encN = encN.reshape(4, 4, P, E).transpose(0, 2, 1, 3)
        in_maps.append({
            "encT": encT8[lo:hi],
            "encB": encTb[lo:hi - 1],
            "wsT": wsT,
            "whT": whT,
            "dhT": np.ascontiguousarray(dhc),
            "vv": vv,
            "amask": np.ascontiguousarray(amask[lo:hi]),
            "encN": np.ascontiguousarray(encN),
        })
    return in_maps


def kernel(decoder_hidden, encoder_outputs, src_mask, W_h, W_s, v, _trace=False):
    if "nc" not in _CACHE:
        _CACHE["nc"] = _build()
    nc = _CACHE["nc"]

    in_maps = _prep_inputs(
        np.asarray(decoder_hidden, dtype=np.float32),
        np.asarray(encoder_outputs, dtype=np.float32),
        np.asarray(src_mask),
        np.asarray(W_h, dtype=np.float32),
        np.asarray(W_s, dtype=np.float32),
        np.asarray(v, dtype=np.float32),
    )

    res = run_bass_kernel_spmd(nc, in_maps, core_ids=list(range(NC)), trace=_trace)
    _CACHE["last_result"] = res

    context = np.empty((B, E), dtype=np.float32)
    attn = np.empty((B, S), dtype=np.float32)
    for c in range(NC):
        lo, hi = c * BPC, (c + 1) * BPC
        attn[lo:hi] = res.results[c]["attn"]
        raw = res.results[c]["ctxr"]  # [P, BPC*ET]
        context[lo:hi] = raw.reshape(P, BPC, ET).transpose(1, 2, 0).reshape(BPC, E)
        # last local batch's context comes from the PE tail fast path
        context[hi - 1] = res.results[c]["ctxl"][0]
    return context, attn


# revision 16
# speedup vs baseline: 1.8559x; 1.0355x over previous
"""Bahdanau additive attention on 8 Trainium2 NeuronCores.

Math (per batch b):
    dec_f  = decoder_hidden @ W_h                     [H]
    enc_f  = encoder_outputs[b] @ W_s                 [S, H]
    energy = tanh(dec_f + enc_f) @ v + addmask        [S]
    attn   = softmax(energy)                          [S]
    context= attn @ encoder_outputs[b]                [2H]

Sharding: data-parallel over batch, 8 batches per core, weights replicated.

Device layout choice: everything runs in "transposed" space — the feature
(contraction) dim of the big matmul lands on SBUF partitions. All DRAM
inputs are pre-swizzled partition-major on the host so every SBUF load is
one contiguous chunk per partition (1 DMA descriptor per partition).

The big matmul runs in fp8 (e4m3) with perf_mode=DoubleRow: the host ships
encT in fp8 and W_s pre-scaled by 256 in fp8 (so its +-1/32 entries use the
normal fp8 range); pairs of contraction tiles fuse into one 256-deep matmul,
halving PE time vs bf16 (measured 216 ns per [256x128]x[256x512] matmul,
the N/2.4GHz streaming floor). The 1/256 descale folds into the tanh's
scale operand. The context weighted-sum needs ~bf16-accurate encoder
values, so a second bf16 copy of encT streams in for the DVE: per batch,
context is a fused multiply+reduce (scalar_tensor_tensor with accum_out;
tensor_tensor_reduce wedges the device) against the SWDGE-broadcast attn
row. enc_f tiles [k=128, s=512] accumulate with W_s pair-tiles stationary
(each stationary serves 2 matmuls); tanh runs on the scalar engine straight
out of PSUM with dec_f as a per-partition bias; energy = v.T @ hidden
accumulates via M=1 matmuls into one of TWO alternating PSUM banks (the 4
s-chunks packed on partitions 0/32/64/96) so a batch never waits on the
previous batch's softmax reads; the 0/-1e10 source mask is added with a
K=1 matmul; softmax runs along the free dim. The previous batch's exp
chunks are emitted one-per-half during the first two k-tiles so they
interleave with tanhs in the scalar engine's strict FIFO instead of
stalling the v-dot matmuls at the batch boundary. Batch b's softmax/
broadcast/context are emitted inside batch b+1's main loop so the in-order
PE queue never waits on the ACT/DVE chain. The last local batch's context
runs on the (by then idle) PE against a natural-layout bf16 copy, with the
attn column vector built by 16 K=1 outer-product matmuls from SBUF (no
DRAM bounce, no transpose DMA).
"""

import numpy as np
import ml_dtypes

import concourse.bacc as bacc
import concourse.mybir as mybir
import concourse.tile as tile
from concourse.bass_utils import run_bass_kernel_spmd

# Problem shapes (hardcoded per contest rules).
B, S, H = 64, 2048, 1024
E = 2 * H            # encoder feature dim
NC = 8               # cores
BPC = B // NC        # batches per core
P = 128              # partitions
ET = E // P          # 16 e-tiles (contraction tiles of main matmul)
PT = ET // 2         # 8 DoubleRow pair-tiles
KT = H // P          # 8 k-tiles (hidden dim tiles)
SC = S // 512        # 4 s-chunks of 512
NEG_BIG = -1e10
WS_SCALE = 256.0     # fp8 pre-scale on W_s (power of 2; descaled in tanh)

F32 = mybir.dt.float32
BF16 = mybir.dt.bfloat16
F8 = mybir.dt.float8e4

_CACHE = {}


def _build():
    nc = bacc.Bacc("TRN2", target_bir_lowering=False, debug=False, num_devices=NC)

    # all layouts partition-major: one contiguous chunk per partition per load
    encT_d = nc.dram_tensor("encT", [BPC, 4, P, 4, S], F8, kind="ExternalInput")
    encB_d = nc.dram_tensor("encB", [BPC - 1, 4, P, 4, S], BF16, kind="ExternalInput")
    # k-major so the k=0 slice can load first (it alone gates batch 0)
    ws_d = nc.dram_tensor("wsT", [P, KT, ET, P], F8, kind="ExternalInput")
    wh_d = nc.dram_tensor("whT", [P, KT, H], BF16, kind="ExternalInput")
    dh_d = nc.dram_tensor("dhT", [P, KT, BPC], BF16, kind="ExternalInput")
    v_d = nc.dram_tensor("vv", [P, KT], BF16, kind="ExternalInput")
    am_d = nc.dram_tensor("amask", [BPC, S], BF16, kind="ExternalInput")
    # natural-layout copy of the LAST local batch (for the tail fast path)
    encN_d = nc.dram_tensor("encN", [4, P, 4, E], BF16, kind="ExternalInput")

    attn_d = nc.dram_tensor("attn", [BPC, S], F32, kind="ExternalOutput")
    ctx_d = nc.dram_tensor("ctxr", [P, BPC * ET], F32, kind="ExternalOutput")
    ctxl_d = nc.dram_tensor("ctxl", [1, E], F32, kind="ExternalOutput")

    with tile.TileContext(nc) as tc:
        with (
            tc.tile_pool(name="const", bufs=1) as cpool,
            tc.tile_pool(name="psum_mm", bufs=6, space="PSUM") as mmp,
            tc.tile_pool(name="psum_en", bufs=2, space="PSUM") as enp,
        ):
            # ---- persistent constants ----
            v_sb = cpool.tile([P, KT], BF16)
            ws_sb = cpool.tile([P, KT, ET, P], F8)
            decf_sb = cpool.tile([P, KT, BPC], F32)
            one_one = cpool.tile([1, 1], BF16)
            nc.vector.memset(one_one[:], 1.0)
            ones_row = cpool.tile([1, P], F32)
            nc.vector.memset(ones_row[:], 1.0)
            # energy lives packed on partitions {0,32,64,96} of one psum
            # bank; ones4 selects those rows in the cross-partition sum
            # matmul, ssum_t holds the per-chunk exp partial sums, and
            # ones4_bf streams the K=1 outer products of the tail's attnT.
            ones4 = cpool.tile([97, 1], F32)
            nc.vector.memset(ones4[:], 0.0)
            ones4_bf = cpool.tile([97, 1], BF16)
            nc.vector.memset(ones4_bf[:], 1.0)
            ssum_t = cpool.tile([97, 1], F32)
            nc.vector.memset(ssum_t[:], 0.0)
            for c in range(SC):
                nc.vector.memset(ones4[32 * c:32 * c + 1, :], 1.0)
            ctx_acc = cpool.tile([P, BPC * ET], F32)
            # last batch's columns are produced by the PE tail path instead
            nc.vector.memset(ctx_acc[:], 0.0)

            # ---- batch pipeline ----
            with (
                tc.tile_pool(name="encp", bufs=8) as encp,
                tc.tile_pool(name="encbp", bufs=5) as encbp,
                tc.tile_pool(name="work", bufs=2) as wkp,
            ):
                # startup: dec_f = (decoder_hidden @ W_h).T -> [k, b].
                # W_h borrows one bf16-stream slot (same 16KB/partition
                # footprint). Small DMAs first so the PE's dec_f matmuls
                # start early.
                nc.sync.dma_start(v_sb[:], v_d.ap())
                wh_sb = encbp.tile([P, KT, H], BF16, tag="encb")
                nc.sync.dma_start(wh_sb[:], wh_d.ap())
                dh_sb = wkp.tile([P, KT, BPC], BF16, tag="dh", bufs=1)
                nc.sync.dma_start(dh_sb[:], dh_d.ap())
                nc.sync.dma_start(ws_sb[:, 0:1], ws_d.ap()[:, 0:1])

                def emit_decf(ks):
                    # dec_f matmuls. PSUM->SBUF copies go on DVE: ACT's
                    # in-order queue already holds batch-0 tanhs that
                    # consume decf_sb.
                    for k in ks:
                        dps = mmp.tile([P, BPC], F32, tag="mm", name=f"dps_{k}")
                        for h in range(KT):
                            nc.tensor.matmul(
                                dps[:],
                                wh_sb[:, h, k * P:(k + 1) * P],
                                dh_sb[:, h, :],
                                start=(h == 0),
                                stop=(h == KT - 1),
                            )
                        nc.vector.tensor_copy(decf_sb[:, k, :], dps[:])

                emit_decf(range(KT))

                state = {}

                def emit_load(b):
                    quarters = []
                    for q in range(4):
                        encq = encp.tile([P, 4, S], F8, tag="encq", name=f"encq_{b}_{q}")
                        nc.sync.dma_start(encq[:], encT_d.ap()[b, q])
                        quarters.append(encq)
                    mask_st = wkp.tile([1, S], BF16, tag="mask", bufs=1, name=f"mask_{b}")
                    nc.sync.dma_start(mask_st[:], am_d.ap()[b:b + 1, :])
                    state[b] = (quarters, mask_st)

                def emit_load_bf(b):
                    # bf16 copy of batch b's encT for the context reduction
                    # (consumed during batch b+1). The last batch skips it:
                    # its context runs on the PE from the natural copy.
                    bfq = []
                    for q in range(4):
                        t = encbp.tile([P, 4, S], BF16, tag="encb", name=f"encb_{b}_{q}")
                        nc.sync.dma_start(t[:], encB_d.ap()[b, q])
                        bfq.append(t)
                    state[b, "bf"] = bfq

                def emit_exp_chunk(b, c):
                    """Exp over one of batch b's energy chunk rows. Chunks
                    are emitted one per half-iteration of batch b+1 so they
                    interleave with tanhs in ACT's strict FIFO instead of
                    blocking the v-dot matmuls at the batch boundary."""
                    energy_ps = state[b, "energy"]
                    if c == 0:
                        exps = wkp.tile([97, 512], F32, tag="exps", bufs=1,
                                        name=f"exps_{b}")
                        # rows between the 4 chunk strips are never written;
                        # zero them so downstream full-height reads are defined
                        nc.vector.memset(exps[:], 0.0)
                        state[b, "exp"] = exps
                    exps = state[b, "exp"]
                    nc.scalar.activation(
                        exps[32 * c:32 * c + 1, :],
                        energy_ps[32 * c:32 * c + 1, :],
                        mybir.ActivationFunctionType.Exp,
                        accum_out=ssum_t[32 * c:32 * c + 1, :],
                    )

                def emit_sum(b, last=False):
                    """Cross-partition exp total + reciprocal for batch b.
                    Emitted one k-tile before emit_post so the PE never waits
                    on the DVE reciprocal between the two tiny matmuls.

                    For the last batch the total accumulates via 4 K=1
                    matmuls, one per chunk: each is gated on that chunk's
                    exp-accumulator drain, so the (otherwise idle) PE stays
                    inside the HAM busy window through the exp chain and the
                    tail context matmuls run at full clock."""
                    sum_ps = mmp.tile([1, 1], F32, tag="mm", name=f"sum_ps_{b}")
                    if last:
                        for c in range(SC):
                            nc.tensor.matmul(
                                sum_ps[:],
                                ssum_t[32 * c:32 * c + 1, 0:1],
                                ones4[32 * c:32 * c + 1, :],
                                start=(c == 0),
                                stop=(c == SC - 1),
                                skip_group_check=True,
                                tile_position=(32 * c, 0),
                            )
                    else:
                        nc.tensor.matmul(sum_ps[:], ssum_t[:], ones4[:],
                                         start=True, stop=True)
                    sinv = wkp.tile([1, 1], F32, tag="sinv", name=f"sinv_{b}")
                    nc.vector.reciprocal(sinv[:], sum_ps[:])
                    state[b, "sinv"] = sinv

                def emit_post(b, last=False):
                    """Softmax tail + attn broadcast + context for batch b.

                    Emitted in the middle of batch b+1's main loop so the PE
                    never stalls waiting on the ACT/DVE softmax chain."""
                    exps = state[b, "exp"]
                    sinv = state[b, "sinv"]
                    # broadcast 1/sum to all partitions with a K=1 matmul
                    sb_ps = mmp.tile([P, 1], F32, tag="mm", name=f"sb_ps_{b}")
                    nc.tensor.matmul(sb_ps[:], ones_row[:], sinv[:],
                                     start=True, stop=True)
                    sinv_all = wkp.tile([P, 1], F32, tag="sinv_all",
                                        name=f"sinv_all_{b}")
                    nc.vector.tensor_copy(sinv_all[:], sb_ps[:])
                    attn_row = wkp.tile([97, 512], F32, tag="attn_row", bufs=1,
                                        name=f"attn_row_{b}")
                    nc.vector.tensor_scalar_mul(attn_row[:], exps[:],
                                                sinv_all[0:97, :])
                    # one DMA for the whole attn row (partition-strided src)
                    nc.sync.dma_start(
                        attn_d.ap()[b:b + 1, :].rearrange("o (c x) -> (o c) x", c=SC),
                        attn_row[0:97:32, :],
                    )

                    if last:
                        # tail fast path: context on the (now idle) PE.
                        # attnT columns are built by 16 K=1 outer-product
                        # matmuls straight from SBUF (exps_bf row segment x
                        # scalar 1) into one PSUM bank -- no DRAM bounce.
                        # The 64 context matmuls then accumulate the
                        # unnormalized context into the packed one-bank
                        # PSUM layout; 1/sum is applied by the scalar
                        # engine during the PSUM->SBUF copy.
                        exps_bf = wkp.tile([97, 512], BF16, tag="exps_bf", bufs=1,
                                           name=f"exps_bf_{b}")
                        nc.vector.tensor_copy(exps_bf[:], exps[:])
                        atps = mmp.tile([P, ET], F32, tag="mm", name="atps")
                        for t in range(ET):
                            c, seg = divmod(t, 4)
                            nc.tensor.matmul(
                                atps[:, t:t + 1],
                                exps_bf[32 * c:32 * c + 1, seg * P:(seg + 1) * P],
                                ones4_bf[32 * c:32 * c + 1, :],
                                start=(t == 0),
                                stop=(t == ET - 1),
                                skip_group_check=True,
                                tile_position=(32 * c, 0),
                            )
                        attnT = wkp.tile([P, ET], BF16, tag="attnT", bufs=1,
                                         name=f"attnT_{b}")
                        nc.vector.tensor_copy(attnT[:], atps[:])
                        natq = state["natq"]
                        ctxps = enp.tile([97, 512], F32, tag="energy",
                                         name="ctxps_last")
                        for t in range(ET):
                            for c in range(SC):
                                nc.tensor.matmul(
                                    ctxps[32 * c:32 * c + 1, :],
                                    attnT[:, t:t + 1],
                                    natq[t // 4][:, t % 4, c * 512:(c + 1) * 512],
                                    start=(t == 0),
                                    stop=(t == ET - 1),
                                    skip_group_check=True,
                                    tile_position=(0, 32 * c),
                                )
                        ctx_row = wkp.tile([97, 512], F32, tag="ctx_row", bufs=1,
                                           name="ctx_row_last")
                        for c in range(SC):
                            nc.scalar.activation(
                                ctx_row[32 * c:32 * c + 1, :],
                                ctxps[32 * c:32 * c + 1, :],
                                mybir.ActivationFunctionType.Copy,
                                scale=sinv_all[32 * c:32 * c + 1, :],
                            )
                        nc.sync.dma_start(
                            ctxl_d.ap()[0:1, :].rearrange("o (c x) -> (o c) x", c=SC),
                            ctx_row[0:97:32, :],
                        )
                        return
                    # pipelined path: broadcast the normalized attn row
                    # from its DRAM output slot (SWDGE: cast in flight)
                    attn_bc = wkp.tile([P, S], BF16, tag="attn_bc", name=f"attn_bc_{b}")
                    nc.gpsimd.dma_start(
                        attn_bc[:], attn_d.ap()[b:b + 1, :].broadcast_to((P, S))
                    )
                    # context: fused multiply+reduce over the bf16 encT tiles
                    bfq = state[b, "bf"]
                    for e in range(ET):
                        q, qt = divmod(e, 4)
                        scr = wkp.tile([P, S], BF16, tag="scr", bufs=2, name=f"scr_{b}_{e}")
                        acc = ctx_acc[:, b * ET + e:b * ET + e + 1]
                        nc.vector.scalar_tensor_tensor(
                            out=scr[:],
                            in0=bfq[q][:, qt, :],
                            scalar=1.0,
                            in1=attn_bc[:],
                            op0=mybir.AluOpType.mult,
                            op1=mybir.AluOpType.mult,
                            accum_out=acc,
                        )

                # batch 0's data next on the DMA queue: its first quarters
                # gate the PE, the rest of W_s only gates k=1 (a k-tile later)
                q0l = []
                for q in range(4):
                    encq = encp.tile([P, 4, S], F8, tag="encq", name=f"encq_0_{q}")
                    nc.sync.dma_start(encq[:], encT_d.ap()[0, q])
                    q0l.append(encq)
                    if q == 1:
                        nc.sync.dma_start(ws_sb[:, 1:KT], ws_d.ap()[:, 1:KT])
                mask0 = wkp.tile([1, S], BF16, tag="mask", bufs=1, name="mask_0")
                nc.sync.dma_start(mask0[:], am_d.ap()[0:1, :])
                state[0] = (q0l, mask0)
                emit_load(1)
                for b in range(BPC):
                    quarters, mask_st = state[b]
                    energy_ps = enp.tile([97, 512], F32, tag="energy", name=f"energy_{b}")
                    state[b, "energy"] = energy_ps

                    # main matmul: enc_f.T tiles + tanh + energy accumulation.
                    # fp8 DoubleRow fuses e-tile pairs into 256-deep matmuls;
                    # s-chunks paired inside the pair-loop so each stationary
                    # W_s pair-tile serves 2 matmuls. Energy matmuls for each
                    # half are deferred one half-iteration so the PE never
                    # waits on the tanh that produces their hidden input.
                    pending = []

                    def flush_pending(keep=0):
                        while len(pending) > keep:
                            k_, sc_, hid_ = pending.pop(0)
                            nc.tensor.matmul(
                                energy_ps[32 * sc_:32 * sc_ + 1, :],
                                v_sb[:, k_:k_ + 1],
                                hid_[:],
                                start=(k_ == 0),
                                stop=False,
                                skip_group_check=True,
                                tile_position=(0, 32 * sc_),
                            )

                    for k in range(KT):
                        if k == 1 and b < BPC - 1:
                            emit_load_bf(b)
                        if k == 2 and b > 0:
                            emit_sum(b - 1)
                        if k == 3 and b > 0:
                            emit_post(b - 1)
                        if k == 4 and b + 2 < BPC:
                            emit_load(b + 2)
                        if k == 3 and b == BPC - 1:
                            natq = []
                            for q in range(4):
                                nq = encbp.tile([P, 4, E], BF16, tag="encb",
                                                name=f"natq_{q}")
                                nc.sync.dma_start(nq[:], encN_d.ap()[q])
                                natq.append(nq)
                            state["natq"] = natq
                        if k == 4 and b == BPC - 1:
                            # the DVE context accumulator is complete once
                            # batch BPC-2's reduction (emitted at k==3) runs
                            nc.sync.dma_start(ctx_d.ap()[:], ctx_acc[:])
                        for half in range(SC // 2):
                            pss = []
                            for j in range(2):
                                sc = half * 2 + j
                                ps = mmp.tile([P, 512], F32, tag="mm",
                                              name=f"ps_{b}_{k}_{sc}")
                                pss.append(ps)
                            for t in range(PT):
                                q, qt = divmod(2 * t, 4)
                                for j in range(2):
                                    sc = half * 2 + j
                                    nc.tensor.matmul(
                                        pss[j][:],
                                        ws_sb[:, k, 2 * t:2 * t + 2, :],
                                        quarters[q][:, qt:qt + 2,
                                                    sc * 512:(sc + 1) * 512],
                                        start=(t == 0),
                                        stop=(t == PT - 1),
                                        perf_mode=mybir.MatmulPerfMode.DoubleRow,
                                    )
                            if half == 0:
                                # flush the previous k-tile's 4 deferred
                                # energy matmuls together: 4 distinct col
                                # groups run concurrently on the PE
                                flush_pending(keep=0)
                            for j in range(2):
                                sc = half * 2 + j
                                hid = wkp.tile([P, 512], BF16, tag="hid", bufs=8,
                                               name=f"hid_{b}_{k}_{sc}")
                                nc.scalar.activation(
                                    hid[:],
                                    pss[j][:],
                                    mybir.ActivationFunctionType.Tanh,
                                    bias=decf_sb[:, k, b:b + 1],
                                    scale=1.0 / WS_SCALE,
                                )
                                pending.append((k, sc, hid))
                            if b > 0 and k < 2:
                                emit_exp_chunk(b - 1, k * 2 + half)
                    flush_pending()
                    # add the (0 / -1e10) mask via a K=1 matmul
                    for sc in range(SC):
                        nc.tensor.matmul(
                            energy_ps[32 * sc:32 * sc + 1, :],
                            one_one[:],
                            mask_st[0:1, sc * 512:(sc + 1) * 512],
                            start=False,
                            stop=True,
                            skip_group_check=True,
                            tile_position=(0, 32 * sc),
                        )
                for c in range(SC):
                    emit_exp_chunk(BPC - 1, c)
                emit_sum(BPC - 1, last=True)
                emit_post(BPC - 1, last=True)

    nc.compile()
    return nc


def _prep_inputs(decoder_hidden, encoder_outputs, src_mask, W_h, W_s, v):
    bf = ml_dtypes.bfloat16
    f8 = ml_dtypes.float8_e4m3
    encoder_outputs = np.asarray(encoder_outputs)
    # [B, E, S] -> partition-major [B, 4(q), 128(p), 4(et), S]
    encT = np.ascontiguousarray(encoder_outputs.transpose(0, 2, 1))
    encT = encT.reshape(B, 4, 4, P, S).transpose(0, 1, 3, 2, 4)
    encT8 = np.ascontiguousarray(encT.astype(f8))
    encTb = np.ascontiguousarray(encT.astype(bf))
    # W_s [E, H] -> k-major [128(p), 8(k), 16(et), 128(m)], pre-scaled for fp8
    wsT = np.ascontiguousarray(
        (W_s * np.float32(WS_SCALE)).reshape(ET, P, KT, P)
        .transpose(1, 2, 0, 3).astype(f8))
    # W_h [H, H] -> [128(p), 8(t), H]
    whT = np.ascontiguousarray(W_h.reshape(KT, P, H).transpose(1, 0, 2).astype(bf))
    # decoder_hidden.T [H, B] -> per-core [128(p), 8(t), BPC] below
    dhT = np.ascontiguousarray(decoder_hidden.T).astype(bf)
    vv = np.ascontiguousarray(v.reshape(KT, P).T).astype(bf)
    amask = np.where(src_mask == 0, np.float32(NEG_BIG), np.float32(0.0)).astype(bf)

    in_maps = []
    for c in range(NC):
        lo, hi = c * BPC, (c + 1) * BPC
        dhc = dhT[:, lo:hi].reshape(KT, P, BPC).transpose(1, 0, 2)
        encN = encoder_outputs[hi - 1].astype(bf)  # [S, E]
        encN = encN.reshape(4, 4, P, E).transpose(0, 2, 1, 3)
        in_maps.append({
            "encT": encT8[lo:hi],
            "encB": encTb[lo:hi - 1],
            "wsT": wsT,
            "whT": whT,
            "dhT": np.ascontiguousarray(dhc),
            "vv": vv,
            "amask": np.ascontiguousarray(amask[lo:hi]),
            "encN": np.ascontiguousarray(encN),
        })
    return in_maps


def kernel(decoder_hidden, encoder_outputs, src_mask, W_h, W_s, v, _trace=False):
    if "nc" not in _CACHE:
        _CACHE["nc"] = _build()
    nc = _CACHE["nc"]

    in_maps = _prep_inputs(
        np.asarray(decoder_hidden, dtype=np.float32),
        np.asarray(encoder_outputs, dtype=np.float32),
        np.asarray(src_mask),
        np.asarray(W_h, dtype=np.float32),
        np.asarray(W_s, dtype=np.float32),
        np.asarray(v, dtype=np.float32),
    )

    res = run_bass_kernel_spmd(nc, in_maps, core_ids=list(range(NC)), trace=_trace)
    _CACHE["last_result"] = res

    context = np.empty((B, E), dtype=np.float32)
    attn = np.empty((B, S), dtype=np.float32)
    for c in range(NC):
        lo, hi = c * BPC, (c + 1) * BPC
        attn[lo:hi] = res.results[c]["attn"]
        raw = res.results[c]["ctxr"]  # [P, BPC*ET]
        context[lo:hi] = raw.reshape(P, BPC, ET).transpose(1, 2, 0).reshape(BPC, E)
        # last local batch's context comes from the PE tail fast path
        context[hi - 1] = res.results[c]["ctxl"][0]
    return context, attn
